# revision 14
# baseline (speedup 1.0000x reference)
"""InterferenceAttention Trainium2 kernel.

Full-input contract: kernel(**inputs) takes the unsharded numpy inputs and
returns the full [B, L, D] output. Internally shards the H=16 heads across
8 NeuronCores (2 heads per core), runs a Bass/Tile kernel SPMD, and
reduces the per-core partial output projections on the host.

Per-core device kernel (h = 2 local heads, L=2048, D=1024, hd=64):
  - load x [L, D] f32, cast bf16, DMA-xbar-transpose to xT [D, L]
  - qT/kT = W x^T (+bias) in [hd*2, L] layout, augmented with 2 phase rows
    so that the rank-2 interference bias rides in the QK^T contraction:
      ST[lk, lq] = sum_d kT_aug[d, lk] * qT_aug[d, lq]
    with kT_aug rows 64,65 = (c, s) and qT_aug rows 64,65 = (g*c, g*s).
  - v in [L, hd] layout with a shared block of 64 ones columns so each
    head's A@V matmul also produces broadcast softmax denominators.
  - scores ST = QK^T in PSUM -> ACT exp -> bf16 -> A@V accumulation.
    (no max-subtraction: scores are O(+-8), exp is safe in f32)
  - oT normalized by DVE reciprocal-multiply, cast bf16.
  - partial = oT^T @ Wo_cols^T streamed PSUM->DRAM.
Host: out = sum_c partial_c + bo.
"""

import numpy as np

import concourse.bass as bass
import concourse.mybir as mybir
import concourse.tile as tile
from concourse import bacc
from concourse.bass_utils import run_bass_kernel_spmd
from concourse.masks import make_identity

# Problem shapes (hardcoded per contract; kernel.py must be self-contained).
B = 1
L = 2048
D = 1024
H = 16
HD = D // H  # 64
BETA = 0.08
EPS = 1e-6

N_CORES = 8
NH = H // N_CORES          # 2 local heads per core
HW = NH * HD               # 128 local head dims per core
LT = L // 128              # 16 L tiles
DT = D // 128              # 8 D chunks

FP32 = mybir.dt.float32
BF16 = mybir.dt.bfloat16
AF = mybir.ActivationFunctionType
ALU = mybir.AluOpType

# Module-level cache: compile once per process.
_NC = None

# Set by test harness: run with NTFF tracing and stash timing here.
TRACE = False
LAST_EXEC_NS = None
LAST_RESULTS = None


def _build():
    nc = bacc.Bacc("TRN2", target_bir_lowering=False, debug=False)

    x_d = nc.dram_tensor("xt", [D, L], FP32, kind="ExternalInput").ap()
    wq_d = nc.dram_tensor("wqt", [D, HW], FP32, kind="ExternalInput").ap()
    wk_d = nc.dram_tensor("wkt", [D, HW], FP32, kind="ExternalInput").ap()
    wv_d = nc.dram_tensor("wvt", [D, HW], FP32, kind="ExternalInput").ap()
    bq_d = nc.dram_tensor("bq", [HW], FP32, kind="ExternalInput").ap()
    bk_d = nc.dram_tensor("bk", [HW], FP32, kind="ExternalInput").ap()
    bv_d = nc.dram_tensor("bv", [HW], FP32, kind="ExternalInput").ap()
    wp_d = nc.dram_tensor("wpt", [D, 2 * NH], FP32, kind="ExternalInput").ap()
    bp_d = nc.dram_tensor("bp", [2 * NH], FP32, kind="ExternalInput").ap()
    gam_d = nc.dram_tensor("gam", [NH], FP32, kind="ExternalInput").ap()
    wo_d = nc.dram_tensor("wot", [HW, D], FP32, kind="ExternalInput").ap()
    out_d = nc.dram_tensor("partial", [L, D], FP32, kind="ExternalOutput").ap()

    with tile.TileContext(nc) as tc:
        _emit(nc, tc, x_d, wq_d, wk_d, wv_d, bq_d, bk_d, bv_d,
              wp_d, bp_d, gam_d, wo_d, out_d)
    nc.compile()
    return nc


def _emit(nc, tc, x_d, wq_d, wk_d, wv_d, bq_d, bk_d, bv_d,
          wp_d, bp_d, gam_d, wo_d, out_d):
    from contextlib import ExitStack
    ctx = ExitStack()
    const = ctx.enter_context(tc.tile_pool(name="const", bufs=1))
    wstage = ctx.enter_context(tc.tile_pool(name="wstage", bufs=2))
    wbf = ctx.enter_context(tc.tile_pool(name="wbf", bufs=1))
    xstage = ctx.enter_context(tc.tile_pool(name="xstage", bufs=3))
    xbfp = ctx.enter_context(tc.tile_pool(name="xbfp", bufs=1))
    xtp = ctx.enter_context(tc.tile_pool(name="xtp", bufs=1))
    qkp = ctx.enter_context(tc.tile_pool(name="qkp", bufs=1))
    php = ctx.enter_context(tc.tile_pool(name="php", bufs=1))
    vp = ctx.enter_context(tc.tile_pool(name="vp", bufs=1))
    expp = ctx.enter_context(tc.tile_pool(name="expp", bufs=3))
    otp = ctx.enter_context(tc.tile_pool(name="otp", bufs=1))
    rp = ctx.enter_context(tc.tile_pool(name="rp", bufs=2))
    ps = ctx.enter_context(tc.tile_pool(name="psum", bufs=1, space="PSUM"))

    # ---- constants ----
    ident_bf = const.tile([128, 128], BF16)
    make_identity(nc, ident_bf)
    ident_f32 = const.tile([128, 128], FP32)
    make_identity(nc, ident_f32)

    bq_sb = const.tile([HW, 1], FP32)
    nc.sync.dma_start(out=bq_sb, in_=bq_d.rearrange("(a b) -> a b", b=1))
    bk_sb = const.tile([HW, 1], FP32)
    nc.sync.dma_start(out=bk_sb, in_=bk_d.rearrange("(a b) -> a b", b=1))
    bp_sb = const.tile([2 * NH, 1], FP32)
    nc.sync.dma_start(out=bp_sb, in_=bp_d.rearrange("(a b) -> a b", b=1))

    # bv broadcast across partitions: bv_bc[p, e] = bv[e]
    bv_bc = const.tile([128, HW], FP32)
    nc.gpsimd.dma_start(
        out=bv_bc,
        in_=bass.AP(tensor=bv_d.tensor, offset=bv_d.offset, ap=[[0, 128], [1, HW]]),
    )
    # gate g = sigmoid(gamma) * BETA, broadcast across partitions: [128, NH]
    g_raw = const.tile([128, NH], FP32)
    nc.gpsimd.dma_start(
        out=g_raw,
        in_=bass.AP(tensor=gam_d.tensor, offset=gam_d.offset, ap=[[0, 128], [1, NH]]),
    )
    g_bc = const.tile([128, NH], FP32)
    nc.scalar.activation(out=g_bc, in_=g_raw, func=AF.Sigmoid)
    nc.vector.tensor_scalar_mul(g_bc, g_bc, BETA)

    # ---- weights (host-fed transposed): load f32, cast bf16 ----
    # wqT/wkT/wvT block j at cols [j*128,(j+1)*128): [d-in-chunk parts, head dim]
    wts = {}
    for name, wdram in (("q", wq_d), ("k", wk_d), ("v", wv_d)):
        st = wstage.tile([128, D], FP32, tag="wst")
        nc.sync.dma_start(
            out=st.rearrange("p (j e) -> p j e", j=DT),
            in_=wdram.rearrange("(j p) e -> p j e", p=128),
        )
        wt = wbf.tile([128, D], BF16, tag=f"w{name}T")
        nc.vector.tensor_copy(out=wt, in_=st)
        wts[name] = wt

    # woT[e, :]: [128 local head dims, D] (host-fed transposed)
    wo_st = wstage.tile([128, D], FP32, tag="wst")
    nc.sync.dma_start(out=wo_st, in_=wo_d)
    woT = wbf.tile([128, D], BF16, tag="woT")
    nc.vector.tensor_copy(out=woT, in_=wo_st)

    # wpT block j at cols [4j, 4j+4): [d-in-chunk parts, 4 phase dims]
    wp_st = wstage.tile([128, 4 * DT], FP32, tag="wpst")
    nc.sync.dma_start(
        out=wp_st.rearrange("p (j c) -> p j c", j=DT),
        in_=wp_d.rearrange("(j p) c -> p j c", p=128),
    )
    wpT = wbf.tile([128, 4 * DT], BF16, tag="wpT")
    nc.vector.tensor_copy(out=wpT, in_=wp_st)

    # ---- x (host-fed transposed): load f32 per d-block, cast bf16 ----
    xT = []
    for dc in range(DT):
        t = xtp.tile([128, L], BF16, tag=f"xT{dc}", name=f"xT{dc}")
        xT.append(t)
    for dc in range(DT):
        st = xstage.tile([128, L], FP32, tag="xst")
        nc.sync.dma_start(out=st, in_=x_d[dc * 128:(dc + 1) * 128, :])
        nc.vector.tensor_copy(out=xT[dc], in_=st)

    # ---- projections: qT_aug / kT_aug [66, L] per head ----
    qa = [qkp.tile([66, L], BF16, tag=f"qa{h}", name=f"qa{h}") for h in range(NH)]
    ka = [qkp.tile([66, L], BF16, tag=f"ka{h}", name=f"ka{h}") for h in range(NH)]

    # ---- phase features ----
    pps_tiles = []
    for cc in range(2):
        pps = ps.tile([4, L // 2], FP32, tag="st", bufs=2, name=f"pps{cc}")
        for dc in range(DT):
            for n in range(2):
                nc.tensor.matmul(
                    pps[:, n * 512:(n + 1) * 512],
                    lhsT=wpT[:, 4 * dc:4 * dc + 4],
                    rhs=xT[dc][:, cc * 1024 + n * 512: cc * 1024 + (n + 1) * 512],
                    start=(dc == 0), stop=(dc == DT - 1),
                )
        pps_tiles.append(pps)
    pT_sb = php.tile([4, L], FP32, tag="pT")
    for cc in range(2):
        nc.vector.tensor_scalar(
            out=pT_sb[:, cc * 1024:(cc + 1) * 1024], in0=pps_tiles[cc],
            scalar1=bp_sb, scalar2=None, op0=ALU.add,
        )
    # transpose to [128, 4*LT]: col 4*lt + r = phase row r of l-tile lt
    pn_ps = ps.tile([128, 4 * LT], FP32, tag="ot", bufs=2)
    for lt in range(LT):
        nc.tensor.transpose(
            out=pn_ps[:, 4 * lt:4 * lt + 4],
            in_=pT_sb[0:4, lt * 128:(lt + 1) * 128],
            identity=ident_f32[0:4, 0:4],
        )
    pn = php.tile([128, 4 * LT], FP32, tag="pn")
    nc.vector.tensor_copy(out=pn, in_=pn_ps)
    sq = php.tile([128, 4 * LT], FP32, tag="sq")
    nc.vector.tensor_mul(sq, pn, pn)

    # per head: c at col offset 2h, s at 2h+1 (stride 4)
    aug_vecs = []  # (target tile, row, [128, LT] bf16 source)
    for h in range(NH):
        c_ap = pn[:, 2 * h::4]
        s_ap = pn[:, 2 * h + 1::4]
        sqc = sq[:, 2 * h::4]
        sqs = sq[:, 2 * h + 1::4]
        n2 = php.tile([128, LT], FP32, tag=f"n2_{h}")
        nc.vector.tensor_tensor(out=n2, in0=sqc, in1=sqs, op=ALU.add)
        nn = php.tile([128, LT], FP32, tag=f"nn_{h}")
        nc.scalar.activation(out=nn, in_=n2, func=AF.Sqrt)
        nc.vector.tensor_scalar_max(nn, nn, EPS)
        rinv = php.tile([128, LT], FP32, tag=f"rinv_{h}")
        nc.vector.reciprocal(out=rinv, in_=nn)
        cn = php.tile([128, LT], FP32, tag=f"cn_{h}")
        nc.vector.tensor_tensor(out=cn, in0=c_ap, in1=rinv, op=ALU.mult)
        sn = php.tile([128, LT], FP32, tag=f"sn_{h}")
        nc.vector.tensor_tensor(out=sn, in0=s_ap, in1=rinv, op=ALU.mult)
        cnb = php.tile([128, LT], BF16, tag=f"cnb_{h}")
        nc.vector.tensor_copy(out=cnb, in_=cn)
        snb = php.tile([128, LT], BF16, tag=f"snb_{h}")
        nc.vector.tensor_copy(out=snb, in_=sn)
        gcb = php.tile([128, LT], BF16, tag=f"gcb_{h}")
        nc.vector.tensor_scalar_mul(gcb, cn, g_bc[:, h:h + 1])
        gsb = php.tile([128, LT], BF16, tag=f"gsb_{h}")
        nc.vector.tensor_scalar_mul(gsb, sn, g_bc[:, h:h + 1])
        aug_vecs += [
            (ka[h], 64, cnb), (ka[h], 65, snb),
            (qa[h], 64, gcb), (qa[h], 65, gsb),
        ]
    for tgt, row, vec in aug_vecs:
        tr = ps.tile([LT, 128], BF16, tag="ot", bufs=2)
        nc.tensor.transpose(out=tr, in_=vec, identity=ident_bf)
        trs = php.tile([LT, 128], BF16, tag="augtr_sb", bufs=2)
        nc.vector.tensor_copy(out=trs, in_=tr)
        nc.sync.dma_start(
            out=tgt[row:row + 1, :].rearrange("a (b c) -> a b c", b=LT),
            in_=trs,
        )

    # ---- projections ----

    for cc in range(2):
        qps = ps.tile([128, L // 2], FP32, tag="st", bufs=2, name=f"qps{cc}")
        for dc in range(DT):
            for n in range(2):
                nc.tensor.matmul(
                    qps[:, n * 512:(n + 1) * 512],
                    lhsT=wts["q"][:, dc * 128:(dc + 1) * 128],
                    rhs=xT[dc][:, cc * 1024 + n * 512: cc * 1024 + (n + 1) * 512],
                    start=(dc == 0), stop=(dc == DT - 1),
                )
        for h in range(NH):
            nc.vector.tensor_scalar(
                out=qa[h][0:HD, cc * 1024:(cc + 1) * 1024],
                in0=qps[h * HD:(h + 1) * HD, :],
                scalar1=bq_sb[h * HD:(h + 1) * HD], scalar2=1.0 / np.sqrt(HD),
                op0=ALU.add, op1=ALU.mult,
            )
    for cc in range(2):
        kps = ps.tile([128, L // 2], FP32, tag="st", bufs=2, name=f"kps{cc}")
        for dc in range(DT):
            for n in range(2):
                nc.tensor.matmul(
                    kps[:, n * 512:(n + 1) * 512],
                    lhsT=wts["k"][:, dc * 128:(dc + 1) * 128],
                    rhs=xT[dc][:, cc * 1024 + n * 512: cc * 1024 + (n + 1) * 512],
                    start=(dc == 0), stop=(dc == DT - 1),
                )
        for h in range(NH):
            nc.vector.tensor_scalar(
                out=ka[h][0:HD, cc * 1024:(cc + 1) * 1024],
                in0=kps[h * HD:(h + 1) * HD, :],
                scalar1=bk_sb[h * HD:(h + 1) * HD], scalar2=None, op0=ALU.add,
            )

    # ---- v tiles: [L-tile, 192] = [v_h0 (64) | ones (64) | v_h1 (64)] ----
    vt = []
    for lt in range(LT):
        t = vp.tile([128, 192], BF16, tag=f"vt{lt}", name=f"vt{lt}")
        nc.vector.memset(t[:, 64:128], 1.0)
        vt.append(t)
    for lt in range(LT):
        vps = ps.tile([128, HW], FP32, tag="ot", bufs=2)
        for dc in range(DT):
            nc.tensor.matmul(
                vps,
                lhsT=xT[dc][:, lt * 128:(lt + 1) * 128],
                rhs=wts["v"][:, dc * 128:(dc + 1) * 128],
                start=(dc == 0), stop=(dc == DT - 1),
            )
        nc.vector.tensor_tensor(
            out=vt[lt][:, 0:64], in0=vps[:, 0:64], in1=bv_bc[:, 0:64], op=ALU.add
        )
        nc.vector.tensor_tensor(
            out=vt[lt][:, 128:192], in0=vps[:, 64:128], in1=bv_bc[:, 64:128],
            op=ALU.add,
        )

    # ---- attention (heads sequential) ----
    oT_sb = otp.tile([128, L], BF16)  # rows h*64..h*64+63 = head h output^T
    for h in range(NH):
        # lhsT: h0 -> [v0 | ones] (cols 0:128), h1 -> [ones | v1] (cols 64:192)
        # h0: psum rows 0:64 = oT, 64:128 = sums
        # h1: psum rows 0:64 = sums, 64:128 = oT
        lo = 0 if h == 0 else 64
        data_rows = (0, 64) if h == 0 else (64, 128)
        sums_rows = (64, 128) if h == 0 else (0, 64)
        for c in range(2):
            oT_ps = ps.tile([128, L // 2], FP32, tag="ot", bufs=2,
                            name=f"oT{h}{c}")
            for lk in range(LT):
                st_ps = ps.tile([128, L // 2], FP32, tag="st", bufs=2,
                                name=f"st{h}{c}{lk}")
                for n in range(2):
                    nc.tensor.matmul(
                        st_ps[:, n * 512:(n + 1) * 512],
                        lhsT=ka[h][:, lk * 128:(lk + 1) * 128],
                        rhs=qa[h][:, c * 1024 + n * 512: c * 1024 + (n + 1) * 512],
                        start=True, stop=True,
                    )
                ex = expp.tile([128, L // 2], BF16, tag="exp", bufs=4)
                nc.scalar.activation(out=ex, in_=st_ps, func=AF.Exp)
                for n in range(2):
                    nc.tensor.matmul(
                        oT_ps[:, n * 512:(n + 1) * 512],
                        lhsT=vt[lk][:, lo:lo + 128],
                        rhs=ex[:, n * 512:(n + 1) * 512],
                        start=(lk == 0), stop=(lk == LT - 1),
                    )
            rv = rp.tile([64, L // 2], FP32, tag="rv", bufs=4)
            nc.vector.reciprocal(out=rv, in_=oT_ps[sums_rows[0]:sums_rows[1], :])
            nc.vector.tensor_tensor(
                out=oT_sb[h * 64:(h + 1) * 64, c * 1024:(c + 1) * 1024],
                in0=oT_ps[data_rows[0]:data_rows[1], :],
                in1=rv, op=ALU.mult,
            )

    # ---- output projection: partial[l, :] = oT^T @ woT ----
    for lt in range(LT):
        for n in range(D // 512):
            op_ps = ps.tile([128, 512], FP32, tag="st", bufs=2)
            nc.tensor.matmul(
                op_ps,
                lhsT=oT_sb[:, lt * 128:(lt + 1) * 128],
                rhs=woT[:, n * 512:(n + 1) * 512],
                start=True, stop=True,
            )
            op_sb = rp.tile([128, 512], FP32, tag="part_sb", bufs=3)
            nc.scalar.activation(out=op_sb, in_=op_ps, func=AF.Copy)
            nc.sync.dma_start(
                out=out_d[lt * 128:(lt + 1) * 128, n * 512:(n + 1) * 512],
                in_=op_sb,
            )
    ctx.close()


def _get_nc():
    global _NC
    if _NC is None:
        _NC = _build()
    return _NC


def kernel(x, Wq, bq, Wk, bk, Wv, bv, Wo, bo, Wp, bp, gamma):
    global LAST_EXEC_NS, LAST_RESULTS
    nc = _get_nc()
    x2 = np.asarray(x, np.float32).reshape(L, D)
    xt = np.ascontiguousarray(x2.T)
    Wq = np.asarray(Wq, np.float32)
    Wk = np.asarray(Wk, np.float32)
    Wv = np.asarray(Wv, np.float32)
    Wo = np.asarray(Wo, np.float32)
    Wp = np.asarray(Wp, np.float32)
    in_maps = []
    for c in range(N_CORES):
        hs = slice(c * HW, (c + 1) * HW)
        ps_ = slice(c * 2 * NH, (c + 1) * 2 * NH)
        in_maps.append({
            "xt": xt,
            "wqt": np.ascontiguousarray(Wq[hs].T),
            "wkt": np.ascontiguousarray(Wk[hs].T),
            "wvt": np.ascontiguousarray(Wv[hs].T),
            "bq": np.ascontiguousarray(np.asarray(bq, np.float32)[hs]),
            "bk": np.ascontiguousarray(np.asarray(bk, np.float32)[hs]),
            "bv": np.ascontiguousarray(np.asarray(bv, np.float32)[hs]),
            "wpt": np.ascontiguousarray(Wp[ps_].T),
            "bp": np.ascontiguousarray(np.asarray(bp, np.float32)[ps_]),
            "gam": np.ascontiguousarray(
                np.asarray(gamma, np.float32)[c * NH:(c + 1) * NH]),
            "wot": np.ascontiguousarray(Wo[:, hs].T),
        })
    res = run_bass_kernel_spmd(nc, in_maps, list(range(N_CORES)), trace=TRACE)
    LAST_EXEC_NS = res.exec_time_ns
    LAST_RESULTS = res
    acc = np.zeros((L, D), np.float32)
    for c in range(N_CORES):
        acc += res.results[c]["partial"]
    acc += np.asarray(bo, np.float32)[None, :]
    return acc.reshape(B, L, D)
